# revision 1
# baseline (speedup 1.0000x reference)
"""Trainium2 Bass kernel for: out_t = silu(cumsum_t(x)) diff along T.

Reference (T, B, L, D) = (4, 2, 2048, 4096) f32:
    Y = silu(cumsum(x, axis=0)); out = concat([Y[:1], Y[1:] - Y[:-1]])

Strategy: shard L across the 8 NeuronCores (embarrassingly parallel; the
scan is over T=4 only).  Per core a raw-Bass 3-stage pipeline streams
chunks of 128x(4x1024) f32 through SBUF:

  SP  : strided 2 MiB HWDGE loads (all 4 t-slices of a chunk at once);
        the first chunk is split into 4 smaller DMAs so all 16 SDMA
        engines ramp up sooner
  DVE : running sums (3 adds) + output diffs (3 subs)
  ACT : 4 silu evaluations (silu0 written straight into the out tile)
        + 2 MiB HWDGE stores on its own ring (GpSimd stays DMA-free);
        the last chunk loads/stores per t-slice to shorten the tail

Explicit semaphores; every dma_start carries zero attached waits (the
DMA ISA encoding only fits one) — cross-engine deps are standalone
sequencer wait_ge instructions.

Compute is f32; the output is stored as bf16 and widened back to f32 on
the host (~2e-3 l2 rel err, well inside the 2e-2 gate), cutting HBM
traffic from 64 MiB to 48 MiB per core: roofline ~141 us at ~358 GB/s,
measured ~160 us (run-to-run +-10% from HBM-stack contention alignment
between paired cores).
"""

import sys

if "/opt/trn_rl_repo" not in sys.path:
    sys.path.insert(0, "/opt/trn_rl_repo")

import numpy as np

T, B, L, D = 4, 2, 2048, 4096
NCORES = 8
LS = L // NCORES            # 256 rows of L per core
NPOS = B * LS * D           # 2_097_152 elements per t-slice per core
P = 128                     # SBUF partitions
F = 1024                    # free-dim elements per tile slice
NCHUNK = NPOS // (P * F)    # 16 chunk iterations per core
NBUF = 5                    # xb / ob slot count
PP = 2                      # acc / y ping-pong depth

_NC_CACHE = {}
LAST_RESULT = None
TRACE = False
TRACE_CORES = None
TMPDIR = None


def _build_nc(use_silu: bool = True):
    import concourse.bass as bass
    from concourse import mybir

    f32 = mybir.dt.float32
    bf16 = mybir.dt.bfloat16
    act_fn = (
        mybir.ActivationFunctionType.Silu
        if use_silu
        else mybir.ActivationFunctionType.Sigmoid
    )

    nc = bass.Bass("TRN2", debug=False)
    # Chunk-major DRAM layout [NCHUNK, P, T, F] (host repacks): each
    # partition's chunk data is one contiguous 16 KiB (load) / 8 KiB
    # (store) run, so every DMA is a straight copy with maximal
    # descriptors — no strided t-permute APs.
    x_d = nc.declare_dram_parameter("x", [NCHUNK, P, T, F], f32, isOutput=False)
    # Output leaves the chip as bf16 (compute stays f32; the host widens
    # back to f32).  Halves store traffic: 32 MiB in + 16 MiB out per
    # core, ~141 us roofline instead of ~187 us, at ~2e-3 rel err.
    o_d = nc.declare_dram_parameter("out", [NCHUNK, P, T, F], bf16, isOutput=True)

    xb = [nc.alloc_sbuf_tensor(f"xb{s}", [P, T, F], f32).ap() for s in range(NBUF)]
    ob = [nc.alloc_sbuf_tensor(f"ob{s}", [P, T, F], bf16).ap() for s in range(NBUF)]
    acc = [
        [nc.alloc_sbuf_tensor(f"acc{p}_{t}", [P, F], f32).ap() for t in range(1, T)]
        for p in range(PP)
    ]
    y = [
        [nc.alloc_sbuf_tensor(f"y{p}_{t}", [P, F], f32).ap() for t in range(1, T)]
        for p in range(PP)
    ]

    import contextlib

    with contextlib.ExitStack() as es:
        block = es.enter_context(nc.Block())
        # One load/store sem lane per buffer slot: a lane's next DMA never
        # overlaps its previous one (slot-reuse waits guarantee it), so the
        # ">= 16*n" threshold semantics stay sound.
        s_load = [es.enter_context(nc.semaphore(f"s_load{k}")) for k in range(NBUF)]
        s_store = [es.enter_context(nc.semaphore(f"s_store{k}")) for k in range(NBUF)]
        s_acc = es.enter_context(nc.semaphore("s_acc"))
        s_act = es.enter_context(nc.semaphore("s_act"))
        s_out = es.enter_context(nc.semaphore("s_out"))
        # Dedicated per-slice sems for the split first-chunk load and the
        # split last-chunk load/store (one DMA per sem keeps every
        # threshold sound).
        s_l0 = [es.enter_context(nc.semaphore(f"s_l0_{t}")) for t in range(T)]
        s_ll = [es.enter_context(nc.semaphore(f"s_ll{t}")) for t in range(T)]
        s_ls = [es.enter_context(nc.semaphore(f"s_ls{t}")) for t in range(T)]
        LAST = NCHUNK - 1

        def ld_lane(i):
            assert i != LAST and i != 0
            return s_load[i % NBUF], 16 * (i // NBUF + (1 if i % NBUF else 0))

        def st_lane(i):
            assert i != LAST
            return s_store[i % NBUF], 16 * (i // NBUF + 1)

        @block.sync
        def _(sp: bass.BassEngine):
            for i in range(NCHUNK):
                if i >= NBUF:
                    j = i - NBUF
                    # xb slot free: DVE adds + ACT silu0 of chunk j done.
                    # (These also transitively cover load j's completion, so
                    # this lane's previous inc is observed before re-use.)
                    sp.wait_ge(s_acc, 3 * (j + 1))
                    sp.wait_ge(s_act, 4 * j + 1)
                if i == 0:
                    # split: smaller first DMAs reach all 16 SDMA engines
                    # (esp. the late-starting ones) sooner
                    for t in range(T):
                        sp.dma_start(
                            out=xb[0][:, t], in_=x_d[0][:, t]
                        ).then_inc(s_l0[t], 16)
                elif i == LAST:
                    # split: per-slice sems let compute start per slice
                    for t in range(T):
                        sp.dma_start(
                            out=xb[i % NBUF][:, t], in_=x_d[i][:, t]
                        ).then_inc(s_ll[t], 16)
                else:
                    sem, _v = ld_lane(i)
                    sp.dma_start(
                        out=xb[i % NBUF][:], in_=x_d[i]
                    ).then_inc(sem, 16)

        @block.vector
        def _(ve: bass.BassEngine):
            def emit_adds(i):
                xs, ps = i % NBUF, i % PP
                a = acc[ps]
                if i == LAST:
                    ve.wait_ge(s_ll[0], 16)
                    ve.wait_ge(s_ll[1], 16)
                elif i == 0:
                    ve.wait_ge(s_l0[0], 16)
                    ve.wait_ge(s_l0[1], 16)
                else:
                    ve.wait_ge(*ld_lane(i))
                if i >= PP:
                    # acc slots free: silus of chunk i-PP done reading them
                    ve.wait_ge(s_act, 4 * (i - PP) + 4)
                ve.tensor_add(a[0][:], xb[xs][:, 0], xb[xs][:, 1]).then_inc(s_acc)
                # same-engine RAW still needs a drain-backed sem wait
                ve.wait_ge(s_acc, 3 * i + 1)
                if i == LAST:
                    ve.wait_ge(s_ll[2], 16)
                elif i == 0:
                    ve.wait_ge(s_l0[2], 16)
                ve.tensor_add(a[1][:], a[0][:], xb[xs][:, 2]).then_inc(s_acc)
                ve.wait_ge(s_acc, 3 * i + 2)
                if i == LAST:
                    ve.wait_ge(s_ll[3], 16)
                elif i == 0:
                    ve.wait_ge(s_l0[3], 16)
                ve.tensor_add(a[2][:], a[1][:], xb[xs][:, 3]).then_inc(s_acc)

            def emit_diffs(i):
                # f32 y tiles -> bf16 ob tile; sub1 reads the bf16 y0 slice
                # ACT wrote into ob directly
                os_, ps = i % NBUF, i % PP
                yy = y[ps]
                if i >= NBUF:
                    ve.wait_ge(*st_lane(i - NBUF))  # ob slot free
                ve.wait_ge(s_act, 4 * i + 2)  # y1 (and ob[:,0]=y0) ready
                ve.tensor_sub(ob[os_][:, 1], yy[0][:], ob[os_][:, 0]).then_inc(s_out)
                ve.wait_ge(s_act, 4 * i + 3)
                ve.tensor_sub(ob[os_][:, 2], yy[1][:], yy[0][:]).then_inc(s_out)
                ve.wait_ge(s_act, 4 * i + 4)
                ve.tensor_sub(ob[os_][:, 3], yy[2][:], yy[1][:]).then_inc(s_out)

            # Software-pipelined order A0, A1, B0, A2, B1, ..., A15, B14,
            # B15: the adds of chunk i+1 run while ACT silus chunk i, so
            # the diffs' s_act waits are already satisfied when reached.
            emit_adds(0)
            for i in range(NCHUNK):
                if i + 1 < NCHUNK:
                    emit_adds(i + 1)
                emit_diffs(i)

        @block.scalar
        def _(se: bass.BassEngine):
            # ACT does the silus AND issues the stores on its own HWDGE ring
            # (qActDynamicHW) — keeps GpSimd DMA-free so the end-of-block
            # dge_drain has nothing to drain.
            for i in range(NCHUNK):
                xs, os_, ps = i % NBUF, i % NBUF, i % PP
                a, yy = acc[ps], y[ps]
                if i == LAST:
                    se.wait_ge(s_ll[0], 16)  # reads xb[:,0]
                elif i == 0:
                    se.wait_ge(s_l0[0], 16)
                else:
                    se.wait_ge(*ld_lane(i))
                if i >= NBUF:
                    se.wait_ge(*st_lane(i - NBUF))  # ob slot free
                if i >= PP:
                    se.wait_ge(s_out, 3 * (i - PP + 1))  # y slots free
                se.activation(ob[os_][:, 0], xb[xs][:, 0], act_fn).then_inc(s_act)
                if i == LAST:
                    # per-slice stores: each output slice leaves as soon as
                    # it's ready, shrinking the end-of-kernel critical path
                    se.wait_ge(s_act, 4 * i + 1)  # own silu0 drained
                    se.dma_start(out=o_d[i][:, 0], in_=ob[os_][:, 0]).then_inc(
                        s_ls[0], 16
                    )
                for t in range(1, T):
                    se.wait_ge(s_acc, 3 * i + t)
                    se.activation(yy[t - 1][:], a[t - 1][:], act_fn).then_inc(s_act)
                if i == LAST:
                    for t in range(1, T):
                        se.wait_ge(s_out, 3 * i + t)
                        se.dma_start(
                            out=o_d[i][:, t], in_=ob[os_][:, t]
                        ).then_inc(s_ls[t], 16)
                else:
                    # store chunk i once DVE's diffs are done
                    se.wait_ge(s_out, 3 * (i + 1))
                    sem, _v = st_lane(i)
                    if i >= NBUF:
                        # observe this lane's previous store before re-inc'ing
                        se.wait_ge(s_store[i % NBUF], 16 * (i // NBUF))
                    se.dma_start(
                        out=o_d[i], in_=ob[i % NBUF][:]
                    ).then_inc(sem, 16)
            for k in range(NBUF):
                n_regular = len([i for i in range(NCHUNK) if i % NBUF == k and i != LAST])
                se.wait_ge(s_store[k], 16 * n_regular)
            for t in range(T):
                se.wait_ge(s_ls[t], 16)

    return nc


def get_nc(use_silu: bool = True):
    key = ("nc", use_silu)
    if key not in _NC_CACHE:
        _NC_CACHE[key] = _build_nc(use_silu)
    return _NC_CACHE[key]


def kernel(x: np.ndarray) -> np.ndarray:
    global LAST_RESULT
    from concourse.bass_utils import run_bass_kernel_spmd

    nc = get_nc()
    x = np.asarray(x, dtype=np.float32)
    # repack each core's shard to the chunk-major [NCHUNK, P, T, F] DRAM
    # layout the kernel uses (contiguous per-partition DMA runs)
    in_maps = [
        {"x": np.ascontiguousarray(
            x[:, :, c * LS : (c + 1) * LS, :]
            .reshape(T, NCHUNK, P, F)
            .transpose(1, 2, 0, 3)
        )}
        for c in range(NCORES)
    ]
    try:
        res = run_bass_kernel_spmd(
            nc, in_maps, list(range(NCORES)), trace=TRACE, tmpdir=TMPDIR,
            trace_cores=TRACE_CORES,
        )
    except Exception:
        # rare transient NRT_EXEC_UNIT_UNRECOVERABLE; the device recovers
        # on the next execution
        res = run_bass_kernel_spmd(
            nc, in_maps, list(range(NCORES)), trace=TRACE, tmpdir=TMPDIR,
            trace_cores=TRACE_CORES,
        )
    LAST_RESULT = res
    outs = [
        np.asarray(res.results[c]["out"], dtype=np.float32)
        .transpose(2, 0, 1, 3)
        .reshape(T, B, LS, D)
        for c in range(NCORES)
    ]
    return np.concatenate(outs, axis=2)



# revision 6
# speedup vs baseline: 1.2287x; 1.2287x over previous
"""Trainium2 Bass kernel for: out_t = silu(cumsum_t(x)) diff along T.

Reference (T, B, L, D) = (4, 2, 2048, 4096) f32:
    Y = silu(cumsum(x, axis=0)); out = concat([Y[:1], Y[1:] - Y[:-1]])

Strategy: shard L across the 8 NeuronCores (embarrassingly parallel; the
scan is over T=4 only).  Per core a raw-Bass 3-stage pipeline streams
chunks of 128x(4x1024) f32 through SBUF:

  SP  : strided 2 MiB HWDGE loads (all 4 t-slices of a chunk at once);
        the first chunk is split into 4 smaller DMAs so all 16 SDMA
        engines ramp up sooner
  DVE : running sums (3 adds) + output diffs (3 subs)
  ACT : 4 silu evaluations (silu0 written straight into the out tile)
        + 2 MiB HWDGE stores on its own ring (GpSimd stays DMA-free);
        the last chunk loads/stores per t-slice to shorten the tail

Explicit semaphores; every dma_start carries zero attached waits (the
DMA ISA encoding only fits one) — cross-engine deps are standalone
sequencer wait_ge instructions.

Both input and output cross HBM as f16 (the host downcasts x and widens
the result back to f32): ~3e-4 l2 rel err, well inside the 2e-2 gate,
cutting HBM traffic from 64 MiB to 32 MiB per core.  Compute stays f32
(DVE/ACT upconvert internally; acc/y tiles are f32).
"""

import sys

if "/opt/trn_rl_repo" not in sys.path:
    sys.path.insert(0, "/opt/trn_rl_repo")

import numpy as np

T, B, L, D = 4, 2, 2048, 4096
NCORES = 8
LS = L // NCORES            # 256 rows of L per core
NPOS = B * LS * D           # 2_097_152 elements per t-slice per core
P = 128                     # SBUF partitions
F = 1024                    # free-dim elements per tile slice
NCHUNK = NPOS // (P * F)    # 16 chunk iterations per core
NBUF = 5                    # xb / ob slot count
PP = 2                      # acc / y ping-pong depth

_NC_CACHE = {}
LAST_RESULT = None
TRACE = False
TRACE_CORES = None
TMPDIR = None


def _build_nc(use_silu: bool = True):
    import concourse.bass as bass
    from concourse import mybir

    f32 = mybir.dt.float32
    f16 = mybir.dt.float16
    act_fn = (
        mybir.ActivationFunctionType.Silu
        if use_silu
        else mybir.ActivationFunctionType.Sigmoid
    )

    nc = bass.Bass("TRN2", debug=False)
    # Chunk-major DRAM layout [NCHUNK, P, T, F] (host repacks): each
    # partition's chunk data is one contiguous 16 KiB (load) / 8 KiB
    # (store) run, so every DMA is a straight copy with maximal
    # descriptors — no strided t-permute APs.
    x_d = nc.declare_dram_parameter("x", [NCHUNK, P, T, F], f16, isOutput=False)
    # Input and output both cross HBM as f16 (host downcasts x / widens
    # the result): 16 MiB in + 16 MiB out per core at ~3e-4 rel err.
    o_d = nc.declare_dram_parameter("out", [NCHUNK, P, T, F], f16, isOutput=True)

    xb = [nc.alloc_sbuf_tensor(f"xb{s}", [P, T, F], f16).ap() for s in range(NBUF)]
    ob = [nc.alloc_sbuf_tensor(f"ob{s}", [P, T, F], f16).ap() for s in range(NBUF)]
    acc = [
        [nc.alloc_sbuf_tensor(f"acc{p}_{t}", [P, F], f32).ap() for t in range(1, T)]
        for p in range(PP)
    ]
    y = [
        [nc.alloc_sbuf_tensor(f"y{p}_{t}", [P, F], f32).ap() for t in range(1, T)]
        for p in range(PP)
    ]

    import contextlib

    with contextlib.ExitStack() as es:
        block = es.enter_context(nc.Block())
        # One load/store sem lane per buffer slot: a lane's next DMA never
        # overlaps its previous one (slot-reuse waits guarantee it), so the
        # ">= 16*n" threshold semantics stay sound.
        s_load = [es.enter_context(nc.semaphore(f"s_load{k}")) for k in range(NBUF)]
        s_store = [es.enter_context(nc.semaphore(f"s_store{k}")) for k in range(NBUF)]
        s_acc = es.enter_context(nc.semaphore("s_acc"))
        s_act = es.enter_context(nc.semaphore("s_act"))
        s_out = es.enter_context(nc.semaphore("s_out"))
        # Dedicated per-slice sems for the split first-chunk load and the
        # split last-chunk load/store (one DMA per sem keeps every
        # threshold sound).
        s_l0 = [es.enter_context(nc.semaphore(f"s_l0_{t}")) for t in range(T)]
        s_ll = [es.enter_context(nc.semaphore(f"s_ll{t}")) for t in range(T)]
        s_ls = [es.enter_context(nc.semaphore(f"s_ls{t}")) for t in range(T)]
        LAST = NCHUNK - 1

        def ld_lane(i):
            assert i != LAST and i != 0
            return s_load[i % NBUF], 16 * (i // NBUF + (1 if i % NBUF else 0))

        def st_lane(i):
            assert i != LAST
            return s_store[i % NBUF], 16 * (i // NBUF + 1)

        @block.sync
        def _(sp: bass.BassEngine):
            for i in range(NCHUNK):
                if i >= NBUF:
                    j = i - NBUF
                    # xb slot free: DVE adds + ACT silu0 of chunk j done.
                    # (These also transitively cover load j's completion, so
                    # this lane's previous inc is observed before re-use.)
                    sp.wait_ge(s_acc, 3 * (j + 1))
                    sp.wait_ge(s_act, 4 * j + 1)
                if i == 0:
                    # split: smaller first DMAs reach all 16 SDMA engines
                    # (esp. the late-starting ones) sooner
                    for t in range(T):
                        sp.dma_start(
                            out=xb[0][:, t], in_=x_d[0][:, t]
                        ).then_inc(s_l0[t], 16)
                elif i == LAST:
                    # split: per-slice sems let compute start per slice
                    for t in range(T):
                        sp.dma_start(
                            out=xb[i % NBUF][:, t], in_=x_d[i][:, t]
                        ).then_inc(s_ll[t], 16)
                else:
                    sem, _v = ld_lane(i)
                    sp.dma_start(
                        out=xb[i % NBUF][:], in_=x_d[i]
                    ).then_inc(sem, 16)

        @block.vector
        def _(ve: bass.BassEngine):
            def emit_adds(i):
                xs, ps = i % NBUF, i % PP
                a = acc[ps]
                if i == LAST:
                    ve.wait_ge(s_ll[0], 16)
                    ve.wait_ge(s_ll[1], 16)
                elif i == 0:
                    ve.wait_ge(s_l0[0], 16)
                    ve.wait_ge(s_l0[1], 16)
                else:
                    ve.wait_ge(*ld_lane(i))
                if i >= PP:
                    # acc slots free: silus of chunk i-PP done reading them
                    ve.wait_ge(s_act, 4 * (i - PP) + 4)
                ve.tensor_add(a[0][:], xb[xs][:, 0], xb[xs][:, 1]).then_inc(s_acc)
                # same-engine RAW still needs a drain-backed sem wait
                ve.wait_ge(s_acc, 3 * i + 1)
                if i == LAST:
                    ve.wait_ge(s_ll[2], 16)
                elif i == 0:
                    ve.wait_ge(s_l0[2], 16)
                ve.tensor_add(a[1][:], a[0][:], xb[xs][:, 2]).then_inc(s_acc)
                ve.wait_ge(s_acc, 3 * i + 2)
                if i == LAST:
                    ve.wait_ge(s_ll[3], 16)
                elif i == 0:
                    ve.wait_ge(s_l0[3], 16)
                ve.tensor_add(a[2][:], a[1][:], xb[xs][:, 3]).then_inc(s_acc)

            def emit_diffs(i):
                # f32 y tiles -> f16 ob tile; sub1 reads the f16 y0 slice
                # ACT wrote into ob directly
                os_, ps = i % NBUF, i % PP
                yy = y[ps]
                if i >= NBUF:
                    ve.wait_ge(*st_lane(i - NBUF))  # ob slot free
                ve.wait_ge(s_act, 4 * i + 2)  # y1 (and ob[:,0]=y0) ready
                ve.tensor_sub(ob[os_][:, 1], yy[0][:], ob[os_][:, 0]).then_inc(s_out)
                ve.wait_ge(s_act, 4 * i + 3)
                ve.tensor_sub(ob[os_][:, 2], yy[1][:], yy[0][:]).then_inc(s_out)
                ve.wait_ge(s_act, 4 * i + 4)
                ve.tensor_sub(ob[os_][:, 3], yy[2][:], yy[1][:]).then_inc(s_out)

            # Software-pipelined order A0, A1, B0, A2, B1, ..., A15, B14,
            # B15: the adds of chunk i+1 run while ACT silus chunk i, so
            # the diffs' s_act waits are already satisfied when reached.
            emit_adds(0)
            for i in range(NCHUNK):
                if i + 1 < NCHUNK:
                    emit_adds(i + 1)
                emit_diffs(i)

        @block.scalar
        def _(se: bass.BassEngine):
            # ACT does the silus AND issues the stores on its own HWDGE ring
            # (qActDynamicHW) — keeps GpSimd DMA-free so the end-of-block
            # dge_drain has nothing to drain.
            for i in range(NCHUNK):
                xs, os_, ps = i % NBUF, i % NBUF, i % PP
                a, yy = acc[ps], y[ps]
                if i == LAST:
                    se.wait_ge(s_ll[0], 16)  # reads xb[:,0]
                elif i == 0:
                    se.wait_ge(s_l0[0], 16)
                else:
                    se.wait_ge(*ld_lane(i))
                if i >= NBUF:
                    se.wait_ge(*st_lane(i - NBUF))  # ob slot free
                if i >= PP:
                    se.wait_ge(s_out, 3 * (i - PP + 1))  # y slots free
                se.activation(ob[os_][:, 0], xb[xs][:, 0], act_fn).then_inc(s_act)
                if i == LAST:
                    # per-slice stores: each output slice leaves as soon as
                    # it's ready, shrinking the end-of-kernel critical path
                    se.wait_ge(s_act, 4 * i + 1)  # own silu0 drained
                    se.dma_start(out=o_d[i][:, 0], in_=ob[os_][:, 0]).then_inc(
                        s_ls[0], 16
                    )
                for t in range(1, T):
                    se.wait_ge(s_acc, 3 * i + t)
                    se.activation(yy[t - 1][:], a[t - 1][:], act_fn).then_inc(s_act)
                if i == LAST:
                    for t in range(1, T):
                        se.wait_ge(s_out, 3 * i + t)
                        se.dma_start(
                            out=o_d[i][:, t], in_=ob[os_][:, t]
                        ).then_inc(s_ls[t], 16)
                else:
                    # store chunk i once DVE's diffs are done
                    se.wait_ge(s_out, 3 * (i + 1))
                    sem, _v = st_lane(i)
                    if i >= NBUF:
                        # observe this lane's previous store before re-inc'ing
                        se.wait_ge(s_store[i % NBUF], 16 * (i // NBUF))
                    se.dma_start(
                        out=o_d[i], in_=ob[i % NBUF][:]
                    ).then_inc(sem, 16)
            for k in range(NBUF):
                n_regular = len([i for i in range(NCHUNK) if i % NBUF == k and i != LAST])
                se.wait_ge(s_store[k], 16 * n_regular)
            for t in range(T):
                se.wait_ge(s_ls[t], 16)

    return nc


def get_nc(use_silu: bool = True):
    key = ("nc", use_silu)
    if key not in _NC_CACHE:
        _NC_CACHE[key] = _build_nc(use_silu)
    return _NC_CACHE[key]


def kernel(x: np.ndarray) -> np.ndarray:
    global LAST_RESULT
    from concourse.bass_utils import run_bass_kernel_spmd

    nc = get_nc()
    x = np.asarray(x, dtype=np.float32).astype(np.float16)
    # repack each core's shard to the chunk-major [NCHUNK, P, T, F] DRAM
    # layout the kernel uses (contiguous per-partition DMA runs)
    in_maps = [
        {"x": np.ascontiguousarray(
            x[:, :, c * LS : (c + 1) * LS, :]
            .reshape(T, NCHUNK, P, F)
            .transpose(1, 2, 0, 3)
        )}
        for c in range(NCORES)
    ]
    try:
        res = run_bass_kernel_spmd(
            nc, in_maps, list(range(NCORES)), trace=TRACE, tmpdir=TMPDIR,
            trace_cores=TRACE_CORES,
        )
    except Exception:
        # rare transient NRT_EXEC_UNIT_UNRECOVERABLE; the device recovers
        # on the next execution
        res = run_bass_kernel_spmd(
            nc, in_maps, list(range(NCORES)), trace=TRACE, tmpdir=TMPDIR,
            trace_cores=TRACE_CORES,
        )
    LAST_RESULT = res
    outs = [
        np.asarray(res.results[c]["out"], dtype=np.float32)
        .transpose(2, 0, 1, 3)
        .reshape(T, B, LS, D)
        for c in range(NCORES)
    ]
    return np.concatenate(outs, axis=2)

